# revision 12
# baseline (speedup 1.0000x reference)
"""DropConnect kernel for Trainium2 (Bass/Tile), 8-core SPMD — fp8 stream.

Problem: Z[b,o] = sum_d X[b,d] * sign(W[d,o]) * Werr[b,d,o] + bias[0,o]*Berr[b,0,o]
Shapes: X [64,1024] f32, W [1024,2048] f32, bias [1,2048] f32,
        Werr [64,1024,2048] f32, Berr [64,1,2048] f32 -> Z [64,2048] f32.

Sharding: over the contraction axis d (1024 = 8 cores x 128). The host
sharding step re-encodes its Werr slice as fp8_e4m3 (bit-exact for the 0/1
Bernoulli masks) in a sample-pair-interleaved layout [32, 128, 4096], so the
per-core HBM stream is 16 MiB instead of 64 MiB and every DMA descriptor is
a 4096B row (the per-engine line-rate sweet spot measured on HW).

Per pair tile on device:
 - HWDGE load (sync/scalar rings alternate) [128, 4096] fp8.
 - One DVE bitwise-XOR against the replicated sign-bit plane of W applies
   sign(W) to the 0/1 masks in-place (u32 lanes, 4 mask bytes per cycle):
   0x00^0x80 = -0.0, 0x38^0x80 = -1.0 — exact.
 - fp8 DoubleRow matmuls: the two K-planes carry the two SAMPLES of the pair
   (distinct ifmap data, so the fp8 double-pump's 2-elems/cycle is fully
   used), and the 128-wide stationary carries Xhi8 at column s and Xlo8 at
   column 64+s (fp8 hi/lo split of X, ~2^-8 combined precision). PSUM rows
   0:64 accumulate the hi partials, rows 64:128 the lo partials; the host
   gather folds them. 4 matmuls per pair tile (~0.9us) sit under the
   ~1.28us/pair DMA period.
 - bias*Berr rides in mid-stream as bf16 (core 0 carries real data, other
   cores zeros) and folds into the live PSUM group via an identity matmul
   into the hi rows.

Output per core: [128, 2048] f32 partial (hi rows 0:64 + lo rows 64:128 of
this core's d-slice contribution); the host gather sums 8 x (hi + lo).

Fallback: if Werr is not 0/1-valued (fp8 would quantize), kernel() routes to
the original f32 cast-DMA program (build_bass_f32) which handles arbitrary
mask values.
"""

import os
import numpy as np
import ml_dtypes

import concourse.bass as bass
import concourse.mybir as mybir
from concourse.tile import TileContext
from concourse import bacc, bass_utils

BF16 = ml_dtypes.bfloat16
FP8 = ml_dtypes.float8_e4m3

B = 64          # batch (samples)
D = 1024        # contraction dim
O = 2048        # output dim
N_CORES = 8
DSL = D // N_CORES   # 128 d-rows per core
NPAIR = B // 2       # sample pairs per core stream
PW = 2 * O           # pair tile width (bytes per partition row in fp8)
NCHUNK = 4           # matmul free-dim chunks (PSUM bank = 512 f32)
CHUNK = O // NCHUNK  # 512
BB_AT = 20           # pair index at which the bias operand starts loading

WERR_BUFS = 12
MASK_BUFS = 5

_CACHE = {}


def build_bass_fp8(sim_init=False):
    nc = bacc.Bacc(trn_type="TRN2", dynamic_dma_scratch_size=32768)

    werr8 = nc.dram_tensor("werr8", (NPAIR, DSL, PW), mybir.dt.float8e4,
                           kind="ExternalInput")
    w8 = nc.dram_tensor("w8", (DSL, O), mybir.dt.float8e4, kind="ExternalInput")
    xc8 = nc.dram_tensor("xc8", (DSL, 128), mybir.dt.float8e4, kind="ExternalInput")
    eyeb = nc.dram_tensor("eyeb", (B, B), mybir.dt.bfloat16, kind="ExternalInput")
    bberr = nc.dram_tensor("bberr", (B, O), mybir.dt.bfloat16, kind="ExternalInput")
    biasb = nc.dram_tensor("biasb", (B, O), mybir.dt.bfloat16, kind="ExternalInput")
    zout = nc.dram_tensor("zout", (128, O), mybir.dt.float32, kind="ExternalOutput")

    with TileContext(nc) as tc:
        with (
            tc.tile_pool(name="const", bufs=1) as cpool,
            tc.tile_pool(name="stream", bufs=WERR_BUFS) as wpool,
            tc.tile_pool(name="mask", bufs=MASK_BUFS) as mpool,
            tc.tile_pool(name="psum", bufs=1, space="PSUM") as ppool,
        ):
            # --- head: w8 halves lead BOTH werr rings (the first XOR depends
            # on it); xc8/eye/bias ride the Pool SWDGE ring out of the way ---
            w8_t = cpool.tile([DSL, O], mybir.dt.float8e4, tag="w8")
            nc.sync.dma_start(out=w8_t[:, 0:O // 2], in_=w8[:, 0:O // 2])
            nc.scalar.dma_start(out=w8_t[:, O // 2:O], in_=w8[:, O // 2:O])
            xc8_t = cpool.tile([DSL, 128], mybir.dt.float8e4, tag="xc8")
            nc.gpsimd.dma_start(out=xc8_t[:], in_=xc8[:, :])
            eye_t = cpool.tile([B, B], mybir.dt.bfloat16, tag="eye")
            nc.gpsimd.dma_start(out=eye_t[:], in_=eyeb[:, :])

            # sign-bit plane of W, replicated across both halves of a pair
            # tile: wsp = (w8 & 0x80) | (w8 & 0x80) << per-half.
            wsp_t = cpool.tile([DSL, PW], mybir.dt.float8e4, tag="wsp")
            if sim_init:
                nc.gpsimd.memset(wsp_t[:], 0.0)
            w8_u = w8_t[:].bitcast(mybir.dt.uint32)
            wsp_u = wsp_t[:].bitcast(mybir.dt.uint32)
            for h in range(2):
                nc.vector.tensor_scalar(
                    out=wsp_u[:, h * (O // 4):(h + 1) * (O // 4)], in0=w8_u,
                    scalar1=0x80808080, scalar2=None,
                    op0=mybir.AluOpType.bitwise_and,
                )

            # xsel one-hot built on device: zero-fill, then two stride-129
            # diagonal copies (k-plane 0 = Xhi8 at col s*128+s, k-plane 1 =
            # Xlo8 at col s*128+64+s).
            xsel_t = cpool.tile([DSL, B * 128], mybir.dt.float8e4, tag="xsel")
            if sim_init:
                nc.gpsimd.memset(xsel_t[:], 0.0)
            xsel_u = xsel_t[:].bitcast(mybir.dt.uint32)
            nc.vector.tensor_scalar(
                out=xsel_u, in0=xsel_u, scalar1=0, scalar2=0,
                op0=mybir.AluOpType.bitwise_and, op1=mybir.AluOpType.bitwise_or,
            )
            nc.vector.tensor_copy(out=xsel_t[:, 0:B * 128:129], in_=xc8_t[:, 0:B])
            nc.vector.tensor_copy(out=xsel_t[:, B:B * 128:129], in_=xc8_t[:, B:128])

            psum_t = ppool.tile([128, O], mybir.dt.float32, tag="acc")

            bberr_t = cpool.tile([B, O], mybir.dt.bfloat16, tag="bberr")
            biasb_t = cpool.tile([B, O], mybir.dt.bfloat16, tag="biasb")
            bterm_t = cpool.tile([B, O], mybir.dt.bfloat16, tag="bterm")

            # --- main streaming loop over sample pairs ---
            for t in range(NPAIR):
                # each pair tile splits across BOTH HWDGE rings so the two
                # rings advance in lockstep (no inter-ring skew at the head)
                werr_t = wpool.tile([DSL, PW], mybir.dt.float8e4, tag="werr")
                nc.sync.dma_start(out=werr_t[:, 0:O], in_=werr8[t][:, 0:O])
                nc.scalar.dma_start(out=werr_t[:, O:PW], in_=werr8[t][:, O:PW])

                masked_t = mpool.tile([DSL, PW], mybir.dt.float8e4, tag="masked")
                nc.vector.tensor_tensor(
                    out=masked_t[:].bitcast(mybir.dt.uint32),
                    in0=werr_t[:].bitcast(mybir.dt.uint32),
                    in1=wsp_t[:].bitcast(mybir.dt.uint32),
                    op=mybir.AluOpType.bitwise_xor,
                )

                # one DoubleRow matmul per output chunk covers BOTH samples:
                # k-plane u = sample 2t+u's masked chunk paired with its
                # 128-wide one-hot (Xhi at col s, Xlo at col 64+s).
                lhsT = xsel_t[:, 2 * t * 128:(2 * t + 2) * 128].rearrange(
                    "p (k m) -> p k m", k=2)
                rhs_k = masked_t[:].rearrange("p (k n) -> p k n", k=2)
                for j in range(NCHUNK):
                    nc.tensor.matmul(
                        psum_t[:, j * CHUNK:(j + 1) * CHUNK],
                        lhsT,
                        rhs_k[:, :, j * CHUNK:(j + 1) * CHUNK],
                        start=(t == 0),
                        stop=(t == NPAIR - 1),
                        perf_mode=mybir.MatmulPerfMode.DoubleRow,
                        skip_group_check=True,
                    )

                if t == BB_AT:
                    # bias operands ride the Pool ring mid-stream; 4096B rows.
                    for h in range(2):
                        cs = slice(h * O // 2, (h + 1) * O // 2)
                        nc.gpsimd.dma_start(out=bberr_t[:, cs], in_=bberr[:, cs])
                        nc.gpsimd.dma_start(out=biasb_t[:, cs], in_=biasb[:, cs])
                if t == BB_AT + 2:
                    nc.vector.tensor_mul(
                        out=bterm_t[:], in0=bberr_t[:], in1=biasb_t[:]
                    )
                if t == BB_AT + 4:
                    # fold bias*Berr into the live PSUM accumulation (hi
                    # rows) via an identity-weight matmul.
                    for j in range(NCHUNK):
                        nc.tensor.matmul(
                            psum_t[0:B, j * CHUNK:(j + 1) * CHUNK],
                            eye_t[:, 0:B],
                            bterm_t[:, j * CHUNK:(j + 1) * CHUNK],
                            start=False,
                            stop=False,
                            skip_group_check=True,
                        )

            # --- epilogue: column-split copies (DVE || ACT), each half
            # stored on its own HWDGE ring as soon as it is staged ---
            zsb_t = cpool.tile([128, O], mybir.dt.float32, tag="zsb")
            nc.vector.tensor_copy(out=zsb_t[:, 0:O // 2], in_=psum_t[:, 0:O // 2])
            nc.sync.dma_start(out=zout[:, 0:O // 2], in_=zsb_t[:, 0:O // 2])
            nc.scalar.copy(out=zsb_t[:, O // 2:O], in_=psum_t[:, O // 2:O])
            nc.scalar.dma_start(out=zout[:, O // 2:O], in_=zsb_t[:, O // 2:O])

    nc.finalize()
    return nc


def _shard_inputs_fp8(X, W, bias, Werr, Berr):
    X = np.asarray(X, dtype=np.float32)
    W = np.asarray(W, dtype=np.float32)
    bias = np.asarray(bias, dtype=np.float32)
    Werr = np.asarray(Werr, dtype=np.float32)
    Berr = np.asarray(Berr, dtype=np.float32)

    # exact fp8 encode of the 0/1 masks (value-compare so ±0.0 both map to 0)
    w8full = np.where(Werr != 0.0, np.uint8(0x38), np.uint8(0)).view(FP8)

    Xhi = X.astype(FP8)
    Xlo = (X - Xhi.astype(np.float32)).astype(FP8)

    bberr0 = Berr[:, 0, :].astype(BF16)
    biasb0 = np.ascontiguousarray(np.broadcast_to(bias, (B, O))).astype(BF16)
    bbz = np.zeros((B, O), BF16)
    eye = np.eye(B, dtype=BF16)

    in_maps = []
    for c in range(N_CORES):
        dsl = slice(c * DSL, (c + 1) * DSL)
        # pair-interleave: [32, 128, 4096] with samples (2t, 2t+1) side by side
        wslice = w8full[:, dsl, :].reshape(NPAIR, 2, DSL, O)
        wpair = np.ascontiguousarray(wslice.transpose(0, 2, 1, 3)).reshape(
            NPAIR, DSL, PW)
        in_maps.append({
            "werr8": wpair,
            "w8": W[dsl, :].astype(FP8),
            "xc8": np.ascontiguousarray(
                np.concatenate([Xhi.T[dsl, :], Xlo.T[dsl, :]], axis=1)),
            "eyeb": eye,
            "bberr": bberr0 if c == 0 else bbz,
            "biasb": biasb0 if c == 0 else bbz,
        })
    return in_maps


# ---------------------------------------------------------------------------
# Fallback path: original f32 cast-DMA kernel (handles non-binary Werr).
# ---------------------------------------------------------------------------

F32_NCHUNK = 4
F32_CHUNK = O // F32_NCHUNK
F32_N_FILL_MM = 3
F32_BB_AT = 56
F32_WERR_BUFS = 12
F32_MASK_BUFS = 6


def build_bass_f32(sim_init=False):
    nc = bacc.Bacc(trn_type="TRN2", dynamic_dma_scratch_size=32768)

    werr = nc.dram_tensor("werr", (B, DSL, O), mybir.dt.float32, kind="ExternalInput")
    wselb = nc.dram_tensor("wselb", (DSL, O), mybir.dt.bfloat16, kind="ExternalInput")
    xc = nc.dram_tensor("xc", (DSL, 128), mybir.dt.bfloat16, kind="ExternalInput")
    bb = nc.dram_tensor("bb", (B, 2 * O), mybir.dt.float32, kind="ExternalInput")
    eye = nc.dram_tensor("eye", (B, B), mybir.dt.bfloat16, kind="ExternalInput")
    zout = nc.dram_tensor("zout", (128, O), mybir.dt.float32, kind="ExternalOutput")

    with TileContext(nc) as tc:
        with (
            tc.tile_pool(name="const", bufs=1) as cpool,
            tc.tile_pool(name="stream", bufs=F32_WERR_BUFS) as wpool,
            tc.tile_pool(name="mask", bufs=F32_MASK_BUFS) as mpool,
            tc.tile_pool(name="psum", bufs=1, space="PSUM") as ppool,
        ):
            wb_t = cpool.tile([DSL, O], mybir.dt.bfloat16, tag="wb")
            nc.sync.dma_start(out=wb_t[:], in_=wselb[:, :])
            xc_t = cpool.tile([DSL, 128], mybir.dt.bfloat16, tag="xc")
            nc.sync.dma_start(out=xc_t[:], in_=xc[:, :])

            xsel_t = cpool.tile([DSL, B * 128], mybir.dt.bfloat16, tag="xsel")
            if sim_init:
                nc.gpsimd.memset(xsel_t[:], 0.0)
            xsel_u = xsel_t[:].bitcast(mybir.dt.uint16)
            nc.vector.tensor_scalar(
                out=xsel_u, in0=xsel_u, scalar1=0, scalar2=0,
                op0=mybir.AluOpType.bitwise_and, op1=mybir.AluOpType.bitwise_or,
            )
            wb_u = wb_t[:].bitcast(mybir.dt.uint16)
            nc.vector.tensor_scalar(
                out=wb_u, in0=wb_u,
                scalar1=0x8000, scalar2=0x3F80,
                op0=mybir.AluOpType.bitwise_and, op1=mybir.AluOpType.bitwise_or,
            )
            nc.vector.tensor_copy(out=xsel_t[:, 0:B * 128:129], in_=xc_t[:, 0:B])
            nc.vector.tensor_copy(out=xsel_t[:, B:B * 128:129], in_=xc_t[:, B:128])

            psum_t = ppool.tile([128, O], mybir.dt.float32, tag="acc")
            warm_ps = ppool.tile([128, F32_CHUNK], mybir.dt.float32, tag="warm_ps")

            bb_t = cpool.tile([B, 2 * O], mybir.dt.float32, tag="bb")
            bterm_t = cpool.tile([B, O], mybir.dt.bfloat16, tag="bterm")
            eye_t = cpool.tile([B, B], mybir.dt.bfloat16, tag="eye")
            nc.sync.dma_start(out=eye_t[:], in_=eye[:, :])

            for b in range(B):
                werr_t = wpool.tile([DSL, O], mybir.dt.bfloat16, tag="werr")
                masked_t = mpool.tile([DSL, O], mybir.dt.bfloat16, tag="masked")
                if b == 0:
                    for j in range(F32_NCHUNK):
                        cs = slice(j * F32_CHUNK, (j + 1) * F32_CHUNK)
                        nc.gpsimd.dma_start(out=werr_t[:, cs], in_=werr[0][:, cs])
                        nc.vector.tensor_mul(
                            out=masked_t[:, cs], in0=werr_t[:, cs], in1=wb_t[:, cs]
                        )
                else:
                    nc.gpsimd.dma_start(out=werr_t[:], in_=werr[b])
                    nc.vector.tensor_mul(out=masked_t[:], in0=werr_t[:], in1=wb_t[:])

                lhsT = xsel_t[:, b * 128:(b + 1) * 128]
                for j in range(F32_NCHUNK):
                    nc.tensor.matmul(
                        psum_t[:, j * F32_CHUNK:(j + 1) * F32_CHUNK],
                        lhsT,
                        masked_t[:, j * F32_CHUNK:(j + 1) * F32_CHUNK],
                        start=(b == 0),
                        stop=(b == B - 1),
                    )
                for _ in range(F32_N_FILL_MM):
                    nc.tensor.matmul(
                        warm_ps[:], xsel_t[:, 0:128], xsel_t[:, 0:F32_CHUNK],
                        start=True, stop=True,
                    )

                if b == F32_BB_AT:
                    nc.scalar.dma_start(out=bb_t[:], in_=bb[:, :])
                    nc.vector.tensor_mul(
                        out=bterm_t[:], in0=bb_t[:, 0:O], in1=bb_t[:, O:2 * O]
                    )
                if b == F32_BB_AT + 1:
                    for j in range(F32_NCHUNK):
                        nc.tensor.matmul(
                            psum_t[0:B, j * F32_CHUNK:(j + 1) * F32_CHUNK],
                            eye_t[:, 0:B],
                            bterm_t[:, j * F32_CHUNK:(j + 1) * F32_CHUNK],
                            start=False,
                            stop=False,
                        )

            zsb_t = cpool.tile([128, O], mybir.dt.float32, tag="zsb")
            nc.vector.tensor_copy(out=zsb_t[:, 0:O // 2], in_=psum_t[:, 0:O // 2])
            nc.sync.dma_start(out=zout[:, 0:O // 2], in_=zsb_t[:, 0:O // 2])
            nc.scalar.copy(out=zsb_t[:, O // 2:O], in_=psum_t[:, O // 2:O])
            nc.scalar.dma_start(out=zout[:, O // 2:O], in_=zsb_t[:, O // 2:O])

    nc.finalize()
    return nc


def _shard_inputs_f32(X, W, bias, Werr, Berr):
    X = np.asarray(X, dtype=np.float32)
    W = np.asarray(W, dtype=np.float32)
    bias = np.asarray(bias, dtype=np.float32)
    Werr = np.asarray(Werr, dtype=np.float32)
    Berr = np.asarray(Berr, dtype=np.float32)

    Xhi = X.astype(BF16)
    Xlo = (X - Xhi.astype(np.float32)).astype(BF16)

    bb0 = np.concatenate(
        [Berr[:, 0, :], np.broadcast_to(bias, (B, O))], axis=1
    ).astype(np.float32)
    bbz = np.concatenate(
        [Berr[:, 0, :], np.zeros((B, O), np.float32)], axis=1
    ).astype(np.float32)

    in_maps = []
    for c in range(N_CORES):
        dsl = slice(c * DSL, (c + 1) * DSL)
        in_maps.append({
            "werr": np.ascontiguousarray(Werr[:, dsl, :]),
            "wselb": W[dsl, :].astype(BF16),
            "xc": np.concatenate([Xhi.T[dsl, :], Xlo.T[dsl, :]], axis=1),
            "bb": bb0 if c == 0 else bbz,
            "eye": np.eye(B, dtype=BF16),
        })
    return in_maps


LAST_RESULT = None


def kernel(X, W, bias, Werr, Berr):
    global LAST_RESULT
    if not int(os.environ.get("DC_TRACE", "0") or "0"):
        # Defensive: a stray BASS_TRACE in the environment would route
        # run_bass_kernel_spmd into the NTFF-profiling path, which needs an
        # axon hook this image may not provide.
        os.environ.setdefault("BASS_NEVER_TRACE", "1")

    Werr = np.asarray(Werr, dtype=np.float32)
    binary = bool(((Werr == 0) | (Werr == 1)).all())

    trace = bool(int(os.environ.get("DC_TRACE", "0") or "0"))
    if binary:
        if "nc8" not in _CACHE:
            _CACHE["nc8"] = build_bass_fp8()
        nc = _CACHE["nc8"]
        in_maps = _shard_inputs_fp8(X, W, bias, Werr, Berr)
        res = bass_utils.run_bass_kernel_spmd(
            nc, in_maps, core_ids=list(range(N_CORES)), trace=trace,
        )
        LAST_RESULT = res
        acc = np.zeros((B, O), dtype=np.float64)
        for c in range(N_CORES):
            z = res.results[c]["zout"]
            acc += z[0:B, :].astype(np.float64)
            acc += z[B:128, :].astype(np.float64)
        return acc.astype(np.float32)

    if "nc32" not in _CACHE:
        _CACHE["nc32"] = build_bass_f32()
    nc = _CACHE["nc32"]
    in_maps = _shard_inputs_f32(X, W, bias, Werr, Berr)
    res = bass_utils.run_bass_kernel_spmd(
        nc, in_maps, core_ids=list(range(N_CORES)), trace=trace,
    )
    LAST_RESULT = res
    acc = np.zeros((B, O), dtype=np.float64)
    for c in range(N_CORES):
        z = res.results[c]["zout"]
        acc += z[0:B, :].astype(np.float64)
        acc += z[B:128, :].astype(np.float64)
    return acc.astype(np.float32)
